# revision 1
# baseline (speedup 1.0000x reference)
"""Conv4d (B=2, Ci=32, Co=64, 16^4 spatial, k=3^4, stride 1, pad 1) on 8
Trainium2 NeuronCores.

Sharding: 8 cores = batch(2) x T-quarters(4). Each core computes
out[64co, 4t, 16d, 16h, 16w] for its (b, t-quarter).

Per-core layout: SBUF x tile [128, 6t*6d*324] where partition group
r in {0..3} holds ci=32 channels of the padded input restricted to the
D-halo window [4r, 4r+6) (plus T halo), planes flattened as 18x18=324.
The 4 partition groups process the 4 output-D-quarters concurrently via
PE row-group tiling (tile_position=(32r, 0)).

Each output (t, d-pair) plane-pair accumulates 81 tap matmuls
(K=32ci, M=64co, N=512=2d*16h*16w) in fp32r (TF32) into one PSUM bank
per row group; epilogue adds bias (DVE/ACT) and DMAs out.
"""
import sys

sys.path.insert(0, "/opt/trn_rl_repo")
import numpy as np

N_CORES = 8
TAPS = [(kt, kd, kh, kw) for kt in range(3) for kd in range(3)
        for kh in range(3) for kw in range(3)]

_NC = None


def _build():
    global _NC
    if _NC is not None:
        return _NC
    import concourse.bacc as bacc
    import concourse.tile as tile
    from concourse import mybir

    f32 = mybir.dt.float32
    f32r = mybir.dt.float32r

    nc = bacc.Bacc("TRN2", debug=False, target_bir_lowering=False,
                   num_devices=N_CORES)
    xq = nc.dram_tensor("xq", [128, 6 * 6 * 324], f32r, kind="ExternalInput")
    wq = nc.dram_tensor("wq", [32, 81 * 64], f32r, kind="ExternalInput")
    bq = nc.dram_tensor("biasq", [64, 1], f32, kind="ExternalInput")
    out = nc.dram_tensor("out", [64, 16384], f32, kind="ExternalOutput")

    with tile.TileContext(nc) as tc:
        with tc.tile_pool(name="xp", bufs=1) as xp, \
             tc.tile_pool(name="wp", bufs=1) as wp, \
             tc.tile_pool(name="op", bufs=6) as op_, \
             tc.tile_pool(name="pp", bufs=8, space="PSUM") as pp:
            xtile = xp.tile([128, 11664], f32r)
            for tf in range(6):
                nc.gpsimd.dma_start(xtile[:, tf * 1944:(tf + 1) * 1944],
                                    xq.ap()[:, tf * 1944:(tf + 1) * 1944])
            # weights replicated into all 4 partition groups straight from
            # the small [32, 5184] DRAM copy (4x 0.66MB reads)
            wtile = wp.tile([128, 5184], f32r)
            for r in range(4):
                nc.gpsimd.dma_start(wtile[32 * r:32 * r + 32, :], wq.ap()[:])
            btile = wp.tile([64, 1], f32)
            nc.gpsimd.dma_start(btile[:], bq.ap()[:])

            xv = xtile.rearrange("p (t d h w) -> p t d h w",
                                 t=6, d=6, h=18, w=18)

            for to in range(4):
                for dp in range(2):
                    ps = [pp.tile([64, 512], f32, tag="ps",
                                  name=f"ps_{to}_{dp}_{r}") for r in range(4)]
                    for i, (kt, kd, kh, kw) in enumerate(TAPS):
                        for r in range(4):
                            rhs = xv[32 * r:32 * r + 32, to + kt,
                                     2 * dp + kd: 2 * dp + kd + 2,
                                     kh:kh + 16, kw:kw + 16]
                            lhsT = wtile[32 * r:32 * r + 32,
                                         i * 64:(i + 1) * 64]
                            nc.tensor.matmul(ps[r][:, :], lhsT, rhs,
                                             start=(i == 0), stop=(i == 80),
                                             tile_position=(32 * r, 0))
                    for r in range(4):
                        o = op_.tile([64, 512], f32, tag="ob",
                                     name=f"o_{to}_{dp}_{r}")
                        if r < 2:
                            nc.vector.tensor_scalar_add(o[:], ps[r][:, :],
                                                        btile[:, 0:1])
                        else:
                            nc.scalar.activation(
                                o[:], ps[r][:, :],
                                mybir.ActivationFunctionType.Identity,
                                bias=btile[:, 0:1])
                        off = to * 4096 + (4 * r + 2 * dp) * 256
                        nc.gpsimd.dma_start(out.ap()[:, off:off + 512], o[:])
    nc.compile()
    _NC = nc
    return nc


def _round_tf32(a):
    b = np.ascontiguousarray(a).view(np.uint32)
    r = (b + np.uint32(0x00000FFF) + ((b >> np.uint32(13)) & np.uint32(1))) \
        & np.uint32(0xFFFFE000)
    return r.view(np.float32)


def _prep_inputs(x, weight, bias):
    x = np.asarray(x, dtype=np.float32)
    weight = np.asarray(weight, dtype=np.float32)
    bias = np.asarray(bias, dtype=np.float32)

    w9 = weight.reshape(64, 32, 81).transpose(2, 1, 0)  # [tap, ci, co]
    warr = np.ascontiguousarray(w9.transpose(1, 0, 2)).reshape(32, 81 * 64)
    wq = _round_tf32(warr)
    bq = bias.reshape(64, 1).astype(np.float32)

    in_maps = []
    for b in range(2):
        xpad = np.pad(x[b], ((0, 0), (1, 1), (1, 1), (1, 1), (1, 1)))
        for tq in range(4):
            xt = xpad[:, 4 * tq:4 * tq + 6]  # [32, 6, 18, 18, 18]
            xqc = np.empty((128, 11664), np.float32)
            for r in range(4):
                xqc[32 * r:32 * r + 32] = \
                    xt[:, :, 4 * r:4 * r + 6].reshape(32, -1)
            in_maps.append({"xq": _round_tf32(xqc), "wq": wq, "biasq": bq})
    return in_maps


def run_spmd(x, weight, bias, trace=False, trace_cores=None, tmpdir=None):
    """Returns (output ndarray, BassKernelResults)."""
    from concourse.bass_utils import run_bass_kernel_spmd
    nc = _build()
    in_maps = _prep_inputs(x, weight, bias)
    res = run_bass_kernel_spmd(nc, in_maps, core_ids=list(range(N_CORES)),
                               trace=trace, trace_cores=trace_cores,
                               tmpdir=tmpdir)
    out = np.empty((2, 64, 16, 16, 16, 16), np.float32)
    for c in range(N_CORES):
        b, tq = c // 4, c % 4
        out[b, :, 4 * tq:4 * tq + 4] = \
            res.results[c]["out"].reshape(64, 4, 16, 16, 16)
    return out, res


def kernel(x, weight, bias):
    out, _ = run_spmd(x, weight, bias)
    return out



# revision 4
# speedup vs baseline: 1.8852x; 1.8852x over previous
"""Conv4d (B=2, Ci=32, Co=64, 16^4 spatial, k=3^4, stride 1, pad 1) on 8
Trainium2 NeuronCores.

Sharding: 8 cores = batch(2) x T-quarters(4). Each core computes
out[64co, 4t, 16d, 16h, 16w] for its (b, t-quarter).

v2 design (vs baseline):
- bf16 x/weights (fp32 PSUM accumulate): halves DMA-in, enables
  standalone-LDWEIGHTS weight reuse (fp32r forbids it).
- Full 128x128 PE array: 4 row groups (D-quarters, K=32ci each) x 2 col
  groups (d-pair within quarter, M=64co at cols 0-63 / 64-127). 8
  concurrent sub-matmuls per tap via tile_position=(32r, 64c).
- Weight reuse: per tap+position one LDWEIGHTS feeds 2 matmuls (the two
  output frames of the phase). Redundant LDWEIGHTS are deleted from the
  legalized module before compile ("surgery").
- 2 phases of 2 output-T frames: PSUM = 8 banks of [128, 512] fp32
  (partitions 0-63 <- col group 0 = d-pair 0, 64-127 <- d-pair 1).
- PE warm-up matmuls at t=0 (no data deps) keep the PE HAM busy during
  the input DMA so real matmuls run at 2.4 GHz from the start.
"""
import sys

sys.path.insert(0, "/opt/trn_rl_repo")
import numpy as np
import ml_dtypes

N_CORES = 8
NWARM = 28
TAPS = [(kt, kd, kh, kw) for kt in range(3) for kd in range(3)
        for kh in range(3) for kw in range(3)]

_NC = None


def _dedupe_ldweights(nc):
    """Remove InstLdweights that reload the identical weights AP at the
    same tile position with no intervening load at that position. Their
    waits/deps are merged into the immediately following instruction
    (the paired matmul)."""
    from concourse import mybir

    removed_total = 0
    for blk in nc.main_func.blocks:
        insts = list(blk.instructions)
        last = {}
        keep = []
        pending = None  # removed LDW whose waits must move to next inst
        for inst in insts:
            if pending is not None:
                # merge removed LDW's sync deps + waits into this inst
                try:
                    inst.merge_dependencies_from(pending)
                except Exception:
                    pass
                psi = pending.sync_info
                if psi is not None and (psi.on_wait or psi.on_update):
                    si = inst.sync_info
                    if si is None:
                        inst.sync_info = mybir.SyncInfo(
                            on_wait=list(psi.on_wait),
                            on_update=list(psi.on_update))
                    else:
                        inst.sync_info = mybir.SyncInfo(
                            on_wait=list(si.on_wait) + list(psi.on_wait),
                            on_update=list(si.on_update) + list(psi.on_update))
                pending = None
            if isinstance(inst, mybir.InstLdweights):
                tp = tuple(inst.tile_position) if inst.tile_position else (0, 0)
                key = str(inst.ins[0])
                if last.get(tp) == key:
                    pending = inst
                    removed_total += 1
                    continue
                last[tp] = key
            keep.append(inst)
        if len(keep) != len(insts):
            blk.instructions = keep
    return removed_total


def _build():
    global _NC
    if _NC is not None:
        return _NC
    import concourse.bacc as bacc
    import concourse.tile as tile
    from concourse import mybir

    f32 = mybir.dt.float32
    bf16 = mybir.dt.bfloat16

    nc = bacc.Bacc("TRN2", debug=False, target_bir_lowering=False,
                   num_devices=N_CORES)
    xq = nc.dram_tensor("xq", [128, 11664], bf16, kind="ExternalInput")
    wq = nc.dram_tensor("wq", [128, 5184], bf16, kind="ExternalInput")
    bq = nc.dram_tensor("biasq", [128, 1], f32, kind="ExternalInput")
    out = nc.dram_tensor("out", [64, 16384], f32, kind="ExternalOutput")

    with tile.TileContext(nc) as tc:
        with tc.tile_pool(name="xp", bufs=1) as xp, \
             tc.tile_pool(name="wp", bufs=1) as wp, \
             tc.tile_pool(name="op", bufs=6) as op_, \
             tc.tile_pool(name="pp", bufs=8, space="PSUM") as pp:
            # --- PE warm-up: no data deps, runs during input DMA ---
            wu_w = wp.tile([32, 64], bf16)
            wu_x = wp.tile([32, 512], bf16)
            nc.vector.memset(wu_w[:], 0.0)
            nc.vector.memset(wu_x[:], 0.0)
            wu_ps = pp.tile([64, 512], f32, tag="ps", name="wu_ps")
            for _ in range(NWARM):
                nc.tensor.matmul(wu_ps[:], wu_w[:], wu_x[:],
                                 start=True, stop=True, tile_position=(0, 0))

            # --- input DMAs, ordered so first-tap deps arrive first ---
            wtile = wp.tile([128, 5184], bf16)
            btile = wp.tile([128, 1], f32)
            xtile = xp.tile([128, 11664], bf16)
            # kt=0 weight block (27 taps * 64 cols)
            nc.gpsimd.dma_start(wtile[:, 0:1728], wq.ap()[:, 0:1728])
            nc.gpsimd.dma_start(btile[:], bq.ap()[:])
            for tf in range(4):
                nc.gpsimd.dma_start(xtile[:, tf * 1944:(tf + 1) * 1944],
                                    xq.ap()[:, tf * 1944:(tf + 1) * 1944])
            nc.gpsimd.dma_start(wtile[:, 1728:5184], wq.ap()[:, 1728:5184])
            for tf in range(4, 6):
                nc.gpsimd.dma_start(xtile[:, tf * 1944:(tf + 1) * 1944],
                                    xq.ap()[:, tf * 1944:(tf + 1) * 1944])

            xv = xtile.rearrange("p (t d h w) -> p t d h w",
                                 t=6, d=6, h=18, w=18)

            for tos in ((0, 1), (2, 3)):
                ps = {}
                for to in tos:
                    for r in range(4):
                        ps[(to, r)] = pp.tile([128, 512], f32, tag="ps",
                                              name=f"ps_{to}_{r}")
                for i, (kt, kd, kh, kw) in enumerate(TAPS):
                    for r in range(4):
                        lhsT = wtile[32 * r:32 * r + 32, i * 64:(i + 1) * 64]
                        for c in range(2):
                            for to in tos:
                                rhs = xv[32 * r:32 * r + 32, to + kt,
                                         2 * c + kd: 2 * c + kd + 2,
                                         kh:kh + 16, kw:kw + 16]
                                nc.tensor.matmul(
                                    ps[(to, r)][64 * c:64 * c + 64, :],
                                    lhsT, rhs,
                                    start=(i == 0), stop=(i == 80),
                                    tile_position=(32 * r, 64 * c))
                for idx, ((to, r), pst) in enumerate(ps.items()):
                    o = op_.tile([128, 512], f32, tag="ob",
                                 name=f"o_{to}_{r}")
                    if idx % 2 == 0:
                        nc.vector.tensor_scalar_add(o[:], pst[:],
                                                    btile[:, 0:1])
                    else:
                        nc.scalar.activation(
                            o[:], pst[:],
                            mybir.ActivationFunctionType.Identity,
                            bias=btile[:, 0:1])
                    for c in range(2):
                        off = to * 4096 + (4 * r + 2 * c) * 256
                        nc.gpsimd.dma_start(out.ap()[:, off:off + 512],
                                            o[64 * c:64 * c + 64, :])
    _dedupe_ldweights(nc)
    nc.compile()
    _NC = nc
    return nc


def _prep_inputs(x, weight, bias):
    x = np.asarray(x, dtype=np.float32)
    weight = np.asarray(weight, dtype=np.float32)
    bias = np.asarray(bias, dtype=np.float32)
    bf16 = ml_dtypes.bfloat16

    w9 = weight.reshape(64, 32, 81).transpose(2, 1, 0)  # [tap, ci, co]
    warr = np.ascontiguousarray(w9.transpose(1, 0, 2)).reshape(32, 81 * 64)
    wq = np.tile(warr, (4, 1)).astype(bf16)  # [128, 5184]
    bq = np.concatenate([bias, bias]).reshape(128, 1).astype(np.float32)

    in_maps = []
    for b in range(2):
        xpad = np.pad(x[b], ((0, 0), (1, 1), (1, 1), (1, 1), (1, 1)))
        for tq in range(4):
            xt = xpad[:, 4 * tq:4 * tq + 6]  # [32, 6, 18, 18, 18]
            xqc = np.empty((128, 11664), np.float32)
            for r in range(4):
                xqc[32 * r:32 * r + 32] = \
                    xt[:, :, 4 * r:4 * r + 6].reshape(32, -1)
            in_maps.append({"xq": xqc.astype(bf16), "wq": wq, "biasq": bq})
    return in_maps


def run_spmd(x, weight, bias, trace=False, trace_cores=None, tmpdir=None):
    """Returns (output ndarray, BassKernelResults)."""
    from concourse.bass_utils import run_bass_kernel_spmd
    nc = _build()
    in_maps = _prep_inputs(x, weight, bias)
    res = run_bass_kernel_spmd(nc, in_maps, core_ids=list(range(N_CORES)),
                               trace=trace, trace_cores=trace_cores,
                               tmpdir=tmpdir)
    out = np.empty((2, 64, 16, 16, 16, 16), np.float32)
    for c in range(N_CORES):
        b, tq = c // 4, c % 4
        out[b, :, 4 * tq:4 * tq + 4] = \
            res.results[c]["out"].reshape(64, 4, 16, 16, 16)
    return out, res


def kernel(x, weight, bias):
    out, _ = run_spmd(x, weight, bias)
    return out


# revision 10
# speedup vs baseline: 2.1074x; 1.1179x over previous
"""Conv4d (B=2, Ci=32, Co=64, 16^4 spatial, k=3^4, stride 1, pad 1) on 8
Trainium2 NeuronCores.

Sharding: 8 cores = batch(2) x T-quarters(4). Each core computes
out[64co, 4t, 16d, 16h, 16w] for its (b, t-quarter).

v2 design (vs baseline):
- bf16 x/weights (fp32 PSUM accumulate): halves DMA-in, enables
  standalone-LDWEIGHTS weight reuse (fp32r forbids it).
- Full 128x128 PE array: 4 row groups (D-quarters, K=32ci each) x 2 col
  groups (d-pair within quarter, M=64co at cols 0-63 / 64-127). 8
  concurrent sub-matmuls per tap via tile_position=(32r, 64c).
- Weight reuse: per tap+position one LDWEIGHTS feeds 2 matmuls (the two
  output frames of the phase). Redundant LDWEIGHTS are deleted from the
  legalized module before compile ("surgery").
- 2 phases of 2 output-T frames: PSUM = 8 banks of [128, 512] fp32
  (partitions 0-63 <- col group 0 = d-pair 0, 64-127 <- d-pair 1).
- PE warm-up matmuls at t=0 (no data deps) keep the PE HAM busy during
  the input DMA so real matmuls run at 2.4 GHz from the start.
"""
import sys

sys.path.insert(0, "/opt/trn_rl_repo")
import numpy as np
import ml_dtypes

N_CORES = 8
NWARM = 14
TAPS = [(kt, kd, kh, kw) for kt in range(3) for kd in range(3)
        for kh in range(3) for kw in range(3)]

_NC = None


def _dedupe_ldweights(nc):
    """Remove InstLdweights that reload the identical weights AP at the
    same tile position with no intervening load at that position. Their
    waits/deps are merged into the immediately following instruction
    (the paired matmul)."""
    from concourse import mybir

    removed_total = 0
    for blk in nc.main_func.blocks:
        insts = list(blk.instructions)
        last = {}
        keep = []
        pending = None  # removed LDW whose waits must move to next inst
        for inst in insts:
            if pending is not None:
                # merge removed LDW's sync deps + waits into this inst
                try:
                    inst.merge_dependencies_from(pending)
                except Exception:
                    pass
                psi = pending.sync_info
                if psi is not None and (psi.on_wait or psi.on_update):
                    si = inst.sync_info
                    if si is None:
                        inst.sync_info = mybir.SyncInfo(
                            on_wait=list(psi.on_wait),
                            on_update=list(psi.on_update))
                    else:
                        inst.sync_info = mybir.SyncInfo(
                            on_wait=list(si.on_wait) + list(psi.on_wait),
                            on_update=list(si.on_update) + list(psi.on_update))
                pending = None
            if isinstance(inst, mybir.InstLdweights):
                tp = tuple(inst.tile_position) if inst.tile_position else (0, 0)
                key = str(inst.ins[0])
                if last.get(tp) == key:
                    pending = inst
                    removed_total += 1
                    continue
                last[tp] = key
            keep.append(inst)
        if len(keep) != len(insts):
            blk.instructions = keep
    return removed_total


def _build():
    global _NC
    if _NC is not None:
        return _NC
    import concourse.bacc as bacc
    import concourse.tile as tile
    from concourse import mybir

    f32 = mybir.dt.float32
    bf16 = mybir.dt.bfloat16

    nc = bacc.Bacc("TRN2", debug=False, target_bir_lowering=False,
                   num_devices=N_CORES)
    xq = nc.dram_tensor("xq", [128, 11664], bf16, kind="ExternalInput")
    wq = nc.dram_tensor("wq", [128, 5184], bf16, kind="ExternalInput")
    bq = nc.dram_tensor("biasq", [128, 1], f32, kind="ExternalInput")
    # out layout: [128, 8192] where partition p = 64*c + co (c = d-pair
    # half), cols = pair_k(8: phase*4+r) x to_half(2) x (dd2, hw256).
    # Host reassembles. 4 KB contiguous DRAM rows per pair-DMA.
    out = nc.dram_tensor("out", [128, 8192], f32, kind="ExternalOutput")

    with tile.TileContext(nc) as tc:
        with tc.tile_pool(name="xp", bufs=1) as xp, \
             tc.tile_pool(name="wp", bufs=1) as wp, \
             tc.tile_pool(name="op", bufs=6) as op_, \
             tc.tile_pool(name="pp", bufs=8, space="PSUM") as pp:
            # --- PE warm-up: full-array MMs (HAM needs high aggregate
            # PE activity), no data deps, runs during input DMA ---
            wu_w = wp.tile([128, 128], bf16)
            wu_x = wp.tile([128, 512], bf16)
            nc.vector.memset(wu_w[:], 0.0)
            nc.vector.memset(wu_x[:], 0.0)
            wu_ps = pp.tile([128, 512], f32, tag="ps", name="wu_ps")
            for _ in range(NWARM):
                nc.tensor.matmul(wu_ps[:], wu_w[:], wu_x[:],
                                 start=True, stop=True, tile_position=(0, 0))

            # --- input DMAs, ordered so first-tap deps arrive first ---
            wtile = wp.tile([128, 5184], bf16)
            btile = wp.tile([128, 1], f32)
            xtile = xp.tile([128, 11664], bf16)
            # kt=0 weight block (27 taps * 64 cols)
            nc.gpsimd.dma_start(wtile[:, 0:1728], wq.ap()[:, 0:1728])
            nc.gpsimd.dma_start(btile[:], bq.ap()[:])
            for tf in range(4):
                nc.gpsimd.dma_start(xtile[:, tf * 1944:(tf + 1) * 1944],
                                    xq.ap()[:, tf * 1944:(tf + 1) * 1944])
            nc.gpsimd.dma_start(wtile[:, 1728:5184], wq.ap()[:, 1728:5184])
            for tf in range(4, 6):
                nc.gpsimd.dma_start(xtile[:, tf * 1944:(tf + 1) * 1944],
                                    xq.ap()[:, tf * 1944:(tf + 1) * 1944])

            xv = xtile.rearrange("p (t d h w) -> p t d h w",
                                 t=6, d=6, h=18, w=18)

            for phase, tos in enumerate(((0, 1), (2, 3))):
                ps = {}
                for to in tos:
                    for r in range(4):
                        ps[(to, r)] = pp.tile([128, 512], f32, tag="ps",
                                              name=f"ps_{to}_{r}")
                for i, (kt, kd, kh, kw) in enumerate(TAPS):
                    for r in range(4):
                        lhsT = wtile[32 * r:32 * r + 32, i * 64:(i + 1) * 64]
                        for c in range(2):
                            for to in tos:
                                rhs = xv[32 * r:32 * r + 32, to + kt,
                                         2 * c + kd: 2 * c + kd + 2,
                                         kh:kh + 16, kw:kw + 16]
                                nc.tensor.matmul(
                                    ps[(to, r)][64 * c:64 * c + 64, :],
                                    lhsT, rhs,
                                    start=(i == 0), stop=(i == 80),
                                    tile_position=(32 * r, 64 * c))
                # epilogue: banks must release in the order phase B's
                # tap-0 matmuls consume them: (to_lo,r0),(to_hi,r0),
                # (to_lo,r1),... -> r-major, vector/scalar in parallel.
                for r in range(4):
                    stg = op_.tile([128, 1024], f32, tag="ob",
                                   name=f"o_{phase}_{r}")
                    for ti, to in enumerate(tos):
                        pst = ps[(to, r)]
                        dst = stg[:, ti * 512:(ti + 1) * 512]
                        if ti == 0:
                            nc.vector.tensor_scalar_add(dst, pst[:],
                                                        btile[:, 0:1])
                        else:
                            nc.scalar.activation(
                                dst, pst[:],
                                mybir.ActivationFunctionType.Identity,
                                bias=btile[:, 0:1])
                    k = phase * 4 + r
                    nc.sync.dma_start(out.ap()[:, k * 1024:(k + 1) * 1024],
                                      stg[:])
    _dedupe_ldweights(nc)
    nc.compile()
    _NC = nc
    return nc


def _prep_inputs(x, weight, bias):
    x = np.asarray(x, dtype=np.float32)
    weight = np.asarray(weight, dtype=np.float32)
    bias = np.asarray(bias, dtype=np.float32)
    bf16 = ml_dtypes.bfloat16

    w9 = weight.reshape(64, 32, 81).transpose(2, 1, 0)  # [tap, ci, co]
    warr = np.ascontiguousarray(w9.transpose(1, 0, 2)).reshape(32, 81 * 64)
    wq = np.tile(warr, (4, 1)).astype(bf16)  # [128, 5184]
    bq = np.concatenate([bias, bias]).reshape(128, 1).astype(np.float32)

    in_maps = []
    for b in range(2):
        xpad = np.pad(x[b], ((0, 0), (1, 1), (1, 1), (1, 1), (1, 1)))
        for tq in range(4):
            xt = xpad[:, 4 * tq:4 * tq + 6]  # [32, 6, 18, 18, 18]
            xqc = np.empty((128, 11664), np.float32)
            for r in range(4):
                xqc[32 * r:32 * r + 32] = \
                    xt[:, :, 4 * r:4 * r + 6].reshape(32, -1)
            in_maps.append({"xq": xqc.astype(bf16), "wq": wq, "biasq": bq})
    return in_maps


def run_spmd(x, weight, bias, trace=False, trace_cores=None, tmpdir=None):
    """Returns (output ndarray, BassKernelResults)."""
    from concourse.bass_utils import run_bass_kernel_spmd
    nc = _build()
    in_maps = _prep_inputs(x, weight, bias)
    res = run_bass_kernel_spmd(nc, in_maps, core_ids=list(range(N_CORES)),
                               trace=trace, trace_cores=trace_cores,
                               tmpdir=tmpdir)
    out = np.empty((2, 64, 16, 16, 16, 16), np.float32)
    for c in range(N_CORES):
        b, tq = c // 4, c % 4
        # [128, 8192]: dims (c2, co64) x (ph2, r4, to2, dd2, hw256);
        # d = 4r + 2c + dd, global to = 2*ph + to
        oc = res.results[c]["out"].reshape(2, 64, 2, 4, 2, 2, 256)
        oc = oc.transpose(1, 2, 4, 3, 0, 5, 6).reshape(64, 4, 16, 16, 16)
        out[b, :, 4 * tq:4 * tq + 4] = oc
    return out, res


def kernel(x, weight, bias):
    out, _ = run_spmd(x, weight, bias)
    return out
